# revision 1
# baseline (speedup 1.0000x reference)
"""LDPC belief-propagation kernel for Trainium2 (8 NeuronCores, data-parallel).

Math (per batch row, H fixed [3,7], 12 edges, check-major edge order):
  lu_e  = ln|tanh(m_e/2)|           = ln(1-z) - ln(1+z),  z = exp(-|m_e|)
  S_c   = sum_{e in check c} lu_e
  d_e   = S_c - lu_e                (== s_upd, <= 0)
  mag_e = -ln tanh(|d_e|/2)         = ln(1+u) - ln(1-u),  u = exp(d_e)
  sgn_e = prod_{e' in c} sign(m_{e'}) * sign(m_e)    (leave-one-out, +-1)
  c2v_e = mag_e * sgn_e
  new_llr_v = llr_v + sum_{c contains v} c2v_{c,v}
  m'_e  = new_llr_v - c2v_e
Only Exp/Ln/Abs/Sign activations -> one ACT table set, no table switches.
Edges of degree-1 variables (e0,e4,e8) carry constant messages == llr: their
lu/sign are computed once; per-iteration transcendentals cover only the 9
dynamic edges, and deg-1 new_llr terms are added only on the last iteration.
Batch is split into chunks so ACT/DVE/GPSIMD/DMA pipeline across chunks.
"""

import numpy as np

_CACHE = {}

NCORES = 8
P = 128      # partitions
CHUNKS = 2   # batch sub-chunks per core (pipeline depth)


def _build(Bc, iters):
    import contextlib

    import concourse.bass as bass
    import concourse.tile as tile
    from concourse import mybir
    from concourse.alu_op_type import AluOpType as Op

    F = mybir.ActivationFunctionType
    W = Bc // P // CHUNKS  # free columns per partition per chunk
    f32 = mybir.dt.float32

    nc = bass.Bass("TRN2", target_bir_lowering=False, debug=False,
                   num_devices=1)
    llr_d = nc.dram_tensor("llr", [Bc, 7], f32, kind="ExternalInput")
    out_d = nc.dram_tensor("out", [Bc, 7], f32, kind="ExternalOutput")

    def sub(t, off, dims):
        a = t[:] if callable(getattr(t, "__getitem__", None)) else t
        return bass.AP(tensor=a.tensor, offset=a.offset + off,
                       ap=[list(a.ap[0])] + [list(d) for d in dims])

    with tile.TileContext(nc) as tc:
        ctx = contextlib.ExitStack()
        with ctx:
            keep = ctx.enter_context(tc.tile_pool(name="keep", bufs=1))
            work = ctx.enter_context(tc.tile_pool(name="work", bufs=2))

            def K(name, k):
                return keep.tile([P, W * k], f32, tag=name, name=name)

            CB = keep.tile([P, 1], f32, tag="CB", name="CB")
            nc.vector.memset(CB, 1e-38)
            CB2 = keep.tile([P, 1], f32, tag="CB2", name="CB2")
            nc.vector.memset(CB2, 0.99999994)

            # per-chunk persistent state
            LLRs = [K(f"LLR{c}", 7) for c in range(CHUNKS)]
            Ms   = [K(f"M{c}", 12) for c in range(CHUNKS)]
            LUs  = [K(f"LU{c}", 12) for c in range(CHUNKS)]
            SGs  = [K(f"SG{c}", 12) for c in range(CHUNKS)]
            NLs  = [K(f"NL{c}", 7) for c in range(CHUNKS)]

            act = nc.scalar.activation
            vec = nc.vector
            gps = nc.gpsimd

            def g12(t):
                return sub(t, 0, [[12, W], [4, 3], [1, 4]])

            def dyn9(t):
                return sub(t, 1, [[12, W], [4, 3], [1, 3]])

            llr_ap = llr_d.ap().rearrange("(c p w) v -> c p (w v)", c=CHUNKS, p=P)
            out_ap = out_d.ap().rearrange("(c p w) v -> c p (w v)", c=CHUNKS, p=P)

            for c in range(CHUNKS):
                LLR, M = LLRs[c], Ms[c]
                nc.sync.dma_start(out=LLR[:], in_=llr_ap[c])
                vec.tensor_copy(sub(M, 0, [[12, W], [1, 4]]),
                                sub(LLR, 0, [[7, W], [2, 4]]))
                vec.tensor_copy(sub(M, 4, [[12, W], [1, 2]]),
                                sub(LLR, 1, [[7, W], [1, 2]]))
                vec.tensor_copy(sub(M, 6, [[12, W], [1, 2]]),
                                sub(LLR, 5, [[7, W], [1, 2]]))
                vec.tensor_copy(sub(M, 8, [[12, W], [1, 4]]),
                                sub(LLR, 3, [[7, W], [1, 4]]))

            for it in range(iters):
                full = (it == 0)
                lastit = (it == iters - 1)
                for c in range(CHUNKS):
                    LLR, M, LU, SG, NL = LLRs[c], Ms[c], LUs[c], SGs[c], NLs[c]
                    # scratch (tag-shared slots rotate across chunk bodies)
                    ZU  = work.tile([P, W * 12], f32, tag="ZU", name="ZU")
                    LPR = work.tile([P, W * 12], f32, tag="LPR", name="LPR")
                    LQS = work.tile([P, W * 12], f32, tag="LQS", name="LQS")
                    T6  = work.tile([P, W * 6], f32, tag="T6", name="T6")
                    S3  = work.tile([P, W * 3], f32, tag="S3", name="S3")
                    G6  = work.tile([P, W * 6], f32, tag="G6", name="G6")
                    G3  = work.tile([P, W * 3], f32, tag="G3", name="G3")
                    DM  = work.tile([P, W * 12], f32, tag="DM", name="DM")
                    SL  = work.tile([P, W * 12], f32, tag="SL", name="SL")
                    CV  = work.tile([P, W * 12], f32, tag="CV", name="CV")
                    TP  = work.tile([P, W * 2], f32, tag="TP", name="TP")

                    sl = (lambda t: t[:]) if full else dyn9
                    # phi1: lu = ln(1-z) - ln(1+z), z = exp(-|m|) clamped < 1
                    act(sl(ZU), sl(M), F.Abs)
                    act(sl(ZU), sl(ZU), F.Exp, scale=-1.0)
                    act(sl(LPR), sl(ZU), F.Ln, bias=1.0)
                    # scale/bias chosen so the argument stays >= 6e-8 even at
                    # z == 1.0 (m == +-0): keeps lu finite and strictly < 0
                    act(sl(LQS), sl(ZU), F.Ln, bias=CB2[:], scale=-0.99999988)
                    vec.tensor_tensor(sl(LU), sl(LQS), sl(LPR), Op.subtract)
                    # sign (+1 at exact zero via tiny bias)
                    act(sl(SG), sl(M), F.Sign, bias=CB[:])

                    # check sums / sign products
                    vec.tensor_tensor(T6[:], sub(LU, 0, [[12, W], [4, 3], [1, 2]]),
                                      sub(LU, 2, [[12, W], [4, 3], [1, 2]]), Op.add)
                    vec.tensor_tensor(S3[:], sub(T6, 0, [[6, W], [2, 3]]),
                                      sub(T6, 1, [[6, W], [2, 3]]), Op.add)
                    gps.tensor_tensor(G6[:], sub(SG, 0, [[12, W], [4, 3], [1, 2]]),
                                      sub(SG, 2, [[12, W], [4, 3], [1, 2]]), Op.mult)
                    gps.tensor_tensor(G3[:], sub(G6, 0, [[6, W], [2, 3]]),
                                      sub(G6, 1, [[6, W], [2, 3]]), Op.mult)

                    slg = g12 if lastit else dyn9
                    slf = (lambda t: t[:]) if lastit else dyn9
                    S3r = sub(S3, 0, [[3, W], [1, 3], [0, 4 if lastit else 3]])
                    G3r = sub(G3, 0, [[3, W], [1, 3], [0, 4 if lastit else 3]])
                    vec.tensor_tensor(slg(DM), S3r, slg(LU), Op.subtract)
                    act(slf(ZU), slf(DM), F.Exp)
                    act(slf(LPR), slf(ZU), F.Ln, bias=1.0)
                    act(slf(LQS), slf(ZU), F.Ln, bias=1.0, scale=-1.0)
                    gps.tensor_tensor(slg(SL), G3r, slg(SG), Op.mult)
                    vec.tensor_tensor(slf(DM), slf(LPR), slf(LQS), Op.subtract)
                    vec.tensor_tensor(slf(CV), slf(DM), slf(SL), Op.mult)

                    # new_llr for feedback vars v2,v5 (pairs), v4, v6
                    vec.tensor_tensor(TP[:], sub(CV, 1, [[12, W], [5, 2]]),
                                      sub(CV, 5, [[12, W], [5, 2]]), Op.add)
                    vec.tensor_tensor(sub(NL, 2, [[7, W], [3, 2]]),
                                      sub(LLR, 2, [[7, W], [3, 2]]),
                                      TP[:], Op.add)
                    vec.tensor_tensor(sub(NL, 4, [[7, W], [2, 2]]),
                                      sub(LLR, 4, [[7, W], [2, 2]]),
                                      sub(CV, 2, [[12, W], [1, 2]]), Op.add)
                    vec.tensor_tensor(sub(NL, 4, [[7, W], [2, 2]]),
                                      sub(NL, 4, [[7, W], [2, 2]]),
                                      sub(CV, 9, [[12, W], [-2, 2]]), Op.add)
                    vec.tensor_tensor(sub(NL, 6, [[7, W], [1, 1]]),
                                      sub(NL, 6, [[7, W], [1, 1]]),
                                      sub(CV, 11, [[12, W], [1, 1]]), Op.add)

                    if lastit:
                        vec.tensor_tensor(sub(NL, 0, [[7, W], [1, 2]]),
                                          sub(LLR, 0, [[7, W], [1, 2]]),
                                          sub(CV, 0, [[12, W], [4, 2]]), Op.add)
                        vec.tensor_tensor(sub(NL, 3, [[7, W], [1, 1]]),
                                          sub(LLR, 3, [[7, W], [1, 1]]),
                                          sub(CV, 8, [[12, W], [1, 1]]), Op.add)
                        nc.sync.dma_start(out=out_ap[c], in_=NL[:])
                    else:
                        # m' = new_llr - c2v for the 9 dynamic edges
                        vec.tensor_tensor(sub(M, 1, [[12, W], [1, 3]]),
                                          sub(NL, 2, [[7, W], [2, 3]]),
                                          sub(CV, 1, [[12, W], [1, 3]]), Op.subtract)
                        vec.tensor_tensor(sub(M, 9, [[12, W], [1, 3]]),
                                          sub(NL, 4, [[7, W], [1, 3]]),
                                          sub(CV, 9, [[12, W], [1, 3]]), Op.subtract)
                        vec.tensor_tensor(sub(M, 5, [[12, W], [1, 1]]),
                                          sub(NL, 2, [[7, W], [1, 1]]),
                                          sub(CV, 5, [[12, W], [1, 1]]), Op.subtract)
                        vec.tensor_tensor(sub(M, 6, [[12, W], [1, 2]]),
                                          sub(NL, 5, [[7, W], [1, 2]]),
                                          sub(CV, 6, [[12, W], [1, 2]]), Op.subtract)

    # walrus on this stack supports a single sync-wait slot per instruction.
    # Tile emits (a) redundant same-engine waits (trivially satisfied by the
    # engine's FIFO program order once the preceding updates have happened)
    # and (b) a kernel-tail SP drain waiting on the whole global clock, where
    # only the output-DMA wait is load-bearing (the per-engine drain + EVSEM
    # butterfly that follows enforces engine completion).  Strip both.
    import bass_rust
    pref = {"EngineType.DVE": "DVE_", "EngineType.Pool": "Pool_",
            "EngineType.Activation": "Activation_", "EngineType.PE": "PE_",
            "EngineType.SP": "SP_"}
    inc = {}
    for b in nc.m.functions[0].blocks:
        for i in b.instructions:
            si = i.sync_info
            if si is None:
                continue
            if len(si.on_wait) > 1:
                if type(i).__name__ == "InstDrain":
                    dma = [w for w in si.on_wait if "DMA" in w.ant_name]
                    keep_w = dma[-1:] if dma else list(si.on_wait)[:1]
                else:
                    p = pref.get(str(i.engine))
                    keep_w = [w for w in si.on_wait
                              if not (p and w.ant_name.startswith(p)
                                      and w.wait_value <= inc.get(w.ant_name, 0))]
                    assert len(keep_w) <= 1, (i.name, [(w.ant_name, w.wait_value) for w in keep_w], {k: inc.get(k) for k in [w.ant_name for w in si.on_wait]})
                i.sync_info = bass_rust.SyncInfo(on_wait=keep_w,
                                                on_update=list(si.on_update))
                si = i.sync_info
            for u in si.on_update:
                if u.update_mode == "sem-inc":
                    inc[u.ant_name] = inc.get(u.ant_name, 0) + u.update_value
    return nc


def kernel(llr, max_iters):
    llr = np.ascontiguousarray(np.asarray(llr), dtype=np.float32)
    iters = int(np.asarray(max_iters))
    B = llr.shape[0]
    if iters <= 0:
        return llr.reshape(B, 1, 7).copy()

    from concourse.bass_utils import run_bass_kernel_spmd

    Bc = B // NCORES
    key = (Bc, iters)
    if key not in _CACHE:
        _CACHE[key] = _build(Bc, iters)
    nc = _CACHE[key]

    flat = llr.reshape(B, 7)
    in_maps = [{"llr": flat[i * Bc:(i + 1) * Bc]} for i in range(NCORES)]
    res = run_bass_kernel_spmd(nc, in_maps, core_ids=list(range(NCORES)))
    out = np.concatenate([np.asarray(r["out"]) for r in res.results], axis=0)
    return out.reshape(B, 1, 7)



# revision 7
# speedup vs baseline: 1.5451x; 1.5451x over previous
"""LDPC belief-propagation kernel for Trainium2 (8 NeuronCores, data-parallel).

Tanh-product formulation (per batch row, H fixed [3,7], 12 edges,
check-major edge order, static edges e0/e4/e8 at slot j=0 of each check):
  tau_e = tanh(m_e / 2)                   (signed; carries the sign)
  p_e   = prod_{i in check, i != e} tau_i (leave-one-out via pair tree)
  c2v_e = 2 atanh(p_e) = ln(1+p) - ln(1-p)   (clamped Ln like atanh)
  new_llr_v = llr_v + sum_{c contains v} c2v_{c,v}
  m'_e  = new_llr_v - c2v_e
This needs only 3 ACT instructions per iteration (Tanh, Ln, Ln) and no
separate sign pipeline: the product keeps the sign, and ln(1+p)-ln(1-p)
is odd in p.  Static-edge tau is computed once (iteration 0); static c2v
terms are only produced on the last iteration for the deg-1 outputs.
Batch is split into chunks so ACT/DVE/Pool/DMA pipeline across chunks.
"""

import numpy as np

_CACHE = {}

NCORES = 8
P = 128      # partitions
CHUNKS = 2   # batch sub-chunks per core (pipeline depth)


def _build(Bc, iters):
    import contextlib

    import concourse.bass as bass
    import concourse.tile as tile
    from concourse import mybir
    from concourse.alu_op_type import AluOpType as Op

    F = mybir.ActivationFunctionType
    W = Bc // P // CHUNKS  # free columns per partition per chunk
    f32 = mybir.dt.float32

    nc = bass.Bass("TRN2", target_bir_lowering=False, debug=False,
                   num_devices=1)
    llr_d = nc.dram_tensor("llr", [Bc, 7], f32, kind="ExternalInput")
    out_d = nc.dram_tensor("out", [Bc, 7], f32, kind="ExternalOutput")

    def sub(t, off, dims):
        a = t[:] if callable(getattr(t, "__getitem__", None)) else t
        return bass.AP(tensor=a.tensor, offset=a.offset + off,
                       ap=[list(a.ap[0])] + [list(d) for d in dims])

    with tile.TileContext(nc) as tc:
        ctx = contextlib.ExitStack()
        with ctx:
            keep = ctx.enter_context(tc.tile_pool(name="keep", bufs=1))
            work = ctx.enter_context(tc.tile_pool(name="work", bufs=2))

            def K(name, k):
                return keep.tile([P, W * k], f32, tag=name, name=name)

            CB2 = keep.tile([P, 1], f32, tag="CB2", name="CB2")
            nc.vector.memset(CB2, 0.99999994)

            # per-chunk persistent state
            LLRs = [K(f"LLR{c}", 7) for c in range(CHUNKS)]
            Ms   = [K(f"M{c}", 12) for c in range(CHUNKS)]
            Ts   = [K(f"T{c}", 12) for c in range(CHUNKS)]
            NLs  = [K(f"NL{c}", 7) for c in range(CHUNKS)]

            act = nc.scalar.activation
            vec = nc.vector
            gps = nc.gpsimd

            def g12(t):
                return sub(t, 0, [[12, W], [4, 3], [1, 4]])

            def dyn9(t):
                return sub(t, 1, [[12, W], [4, 3], [1, 3]])

            llr_ap = llr_d.ap().rearrange("(c p w) v -> c p (w v)", c=CHUNKS, p=P)
            out_ap = out_d.ap().rearrange("(c p w) v -> c p (w v)", c=CHUNKS, p=P)

            for c in range(CHUNKS):
                LLR, M = LLRs[c], Ms[c]
                nc.sync.dma_start(out=LLR[:], in_=llr_ap[c])
                vec.tensor_copy(sub(M, 0, [[12, W], [1, 4]]),
                                sub(LLR, 0, [[7, W], [2, 4]]))
                vec.tensor_copy(sub(M, 4, [[12, W], [1, 2]]),
                                sub(LLR, 1, [[7, W], [1, 2]]))
                vec.tensor_copy(sub(M, 6, [[12, W], [1, 2]]),
                                sub(LLR, 5, [[7, W], [1, 2]]))
                vec.tensor_copy(sub(M, 8, [[12, W], [1, 4]]),
                                sub(LLR, 3, [[7, W], [1, 4]]))

            for it in range(iters):
                full = (it == 0)
                lastit = (it == iters - 1)
                for c in range(CHUNKS):
                    LLR, M, T, NL = LLRs[c], Ms[c], Ts[c], NLs[c]
                    # scratch (tag-shared slots rotate across chunk bodies)
                    P6  = work.tile([P, W * 6], f32, tag="P6", name="P6")
                    CVP = work.tile([P, W * 12], f32, tag="CVP", name="CVP")
                    LP  = work.tile([P, W * 12], f32, tag="LP", name="LP")
                    LM  = work.tile([P, W * 12], f32, tag="LM", name="LM")
                    CV  = work.tile([P, W * 12], f32, tag="CV", name="CV")
                    TP  = work.tile([P, W * 2], f32, tag="TP", name="TP")

                    sl = (lambda t: t[:]) if full else dyn9
                    # tau = tanh(m/2), signed (static slots only it 0)
                    act(sl(T), sl(M), F.Tanh, scale=0.5)

                    # pair products per check: A = t0*t1, B = t2*t3
                    gps.tensor_tensor(P6[:],
                                      sub(T, 0, [[12, W], [4, 3], [2, 2]]),
                                      sub(T, 1, [[12, W], [4, 3], [2, 2]]),
                                      Op.mult)
                    # leave-one-out products
                    # j=1: p1 = t0 * B
                    gps.tensor_tensor(sub(CVP, 1, [[12, W], [4, 3]]),
                                      sub(T, 0, [[12, W], [4, 3]]),
                                      sub(P6, 1, [[6, W], [2, 3]]),
                                      Op.mult)
                    # j=2,3: {p2,p3} = A * {t3,t2}
                    gps.tensor_tensor(sub(CVP, 2, [[12, W], [4, 3], [1, 2]]),
                                      sub(P6, 0, [[6, W], [2, 3], [0, 2]]),
                                      sub(T, 3, [[12, W], [4, 3], [-1, 2]]),
                                      Op.mult)
                    if lastit:
                        # j=0: p0 = t1 * B (deg-1 outputs need static c2v)
                        gps.tensor_tensor(sub(CVP, 0, [[12, W], [4, 3]]),
                                          sub(T, 1, [[12, W], [4, 3]]),
                                          sub(P6, 1, [[6, W], [2, 3]]),
                                          Op.mult)

                    slf = g12 if lastit else dyn9
                    # c2v = ln(1+p) - ln(1-p); scale/bias keep the Ln
                    # argument >= 6e-8 even at p == +-1.0
                    act(slf(LP), slf(CVP), F.Ln,
                        bias=CB2[:], scale=0.99999988)
                    act(slf(LM), slf(CVP), F.Ln,
                        bias=CB2[:], scale=-0.99999988)
                    vec.tensor_tensor(slf(CV), slf(LP), slf(LM), Op.subtract)

                    # new_llr for feedback vars v2,v5 (pairs), v4, v6
                    vec.tensor_tensor(TP[:], sub(CV, 1, [[12, W], [5, 2]]),
                                      sub(CV, 5, [[12, W], [5, 2]]), Op.add)
                    vec.tensor_tensor(sub(NL, 2, [[7, W], [3, 2]]),
                                      sub(LLR, 2, [[7, W], [3, 2]]),
                                      TP[:], Op.add)
                    vec.tensor_tensor(sub(NL, 4, [[7, W], [2, 2]]),
                                      sub(LLR, 4, [[7, W], [2, 2]]),
                                      sub(CV, 2, [[12, W], [1, 2]]), Op.add)
                    vec.tensor_tensor(sub(NL, 4, [[7, W], [2, 2]]),
                                      sub(NL, 4, [[7, W], [2, 2]]),
                                      sub(CV, 9, [[12, W], [-2, 2]]), Op.add)
                    vec.tensor_tensor(sub(NL, 6, [[7, W], [1, 1]]),
                                      sub(NL, 6, [[7, W], [1, 1]]),
                                      sub(CV, 11, [[12, W], [1, 1]]), Op.add)

                    if lastit:
                        vec.tensor_tensor(sub(NL, 0, [[7, W], [1, 2]]),
                                          sub(LLR, 0, [[7, W], [1, 2]]),
                                          sub(CV, 0, [[12, W], [4, 2]]), Op.add)
                        vec.tensor_tensor(sub(NL, 3, [[7, W], [1, 1]]),
                                          sub(LLR, 3, [[7, W], [1, 1]]),
                                          sub(CV, 8, [[12, W], [1, 1]]), Op.add)
                        nc.sync.dma_start(out=out_ap[c], in_=NL[:])
                    else:
                        # m' = new_llr - c2v for the 9 dynamic edges
                        vec.tensor_tensor(sub(M, 1, [[12, W], [1, 3]]),
                                          sub(NL, 2, [[7, W], [2, 3]]),
                                          sub(CV, 1, [[12, W], [1, 3]]), Op.subtract)
                        vec.tensor_tensor(sub(M, 9, [[12, W], [1, 3]]),
                                          sub(NL, 4, [[7, W], [1, 3]]),
                                          sub(CV, 9, [[12, W], [1, 3]]), Op.subtract)
                        vec.tensor_tensor(sub(M, 5, [[12, W], [1, 1]]),
                                          sub(NL, 2, [[7, W], [1, 1]]),
                                          sub(CV, 5, [[12, W], [1, 1]]), Op.subtract)
                        vec.tensor_tensor(sub(M, 6, [[12, W], [1, 2]]),
                                          sub(NL, 5, [[7, W], [1, 2]]),
                                          sub(CV, 6, [[12, W], [1, 2]]), Op.subtract)

    # walrus on this stack supports a single sync-wait slot per instruction.
    # Tile emits (a) redundant same-engine waits (trivially satisfied by the
    # engine's FIFO program order once the preceding updates have happened)
    # and (b) a kernel-tail SP drain waiting on the whole global clock, where
    # only the output-DMA wait is load-bearing (the per-engine drain + EVSEM
    # butterfly that follows enforces engine completion).  Strip both.
    import bass_rust
    pref = {"EngineType.DVE": "DVE_", "EngineType.Pool": "Pool_",
            "EngineType.Activation": "Activation_", "EngineType.PE": "PE_",
            "EngineType.SP": "SP_"}
    inc = {}
    for b in nc.m.functions[0].blocks:
        for i in b.instructions:
            si = i.sync_info
            if si is None:
                continue
            if len(si.on_wait) > 1:
                if type(i).__name__ == "InstDrain":
                    dma = [w for w in si.on_wait if "DMA" in w.ant_name]
                    keep_w = dma[-1:] if dma else list(si.on_wait)[:1]
                else:
                    p = pref.get(str(i.engine))
                    keep_w = [w for w in si.on_wait
                              if not (p and w.ant_name.startswith(p)
                                      and w.wait_value <= inc.get(w.ant_name, 0))]
                    assert len(keep_w) <= 1, (i.name, [(w.ant_name, w.wait_value) for w in keep_w], {k: inc.get(k) for k in [w.ant_name for w in si.on_wait]})
                i.sync_info = bass_rust.SyncInfo(on_wait=keep_w,
                                                on_update=list(si.on_update))
                si = i.sync_info
            for u in si.on_update:
                if u.update_mode == "sem-inc":
                    inc[u.ant_name] = inc.get(u.ant_name, 0) + u.update_value
    return nc


def kernel(llr, max_iters):
    llr = np.ascontiguousarray(np.asarray(llr), dtype=np.float32)
    iters = int(np.asarray(max_iters))
    B = llr.shape[0]
    if iters <= 0:
        return llr.reshape(B, 1, 7).copy()

    from concourse.bass_utils import run_bass_kernel_spmd

    Bc = B // NCORES
    key = (Bc, iters)
    if key not in _CACHE:
        _CACHE[key] = _build(Bc, iters)
    nc = _CACHE[key]

    flat = llr.reshape(B, 7)
    in_maps = [{"llr": flat[i * Bc:(i + 1) * Bc]} for i in range(NCORES)]
    res = run_bass_kernel_spmd(nc, in_maps, core_ids=list(range(NCORES)))
    out = np.concatenate([np.asarray(r["out"]) for r in res.results], axis=0)
    return out.reshape(B, 1, 7)
